# revision 27
# baseline (speedup 1.0000x reference)
"""Trainium2 Bass kernel for MultiHeadRelativeSelfAttention (Transformer-XL).

Sharding: data-parallel over batch; 8 cores x 1 batch element, no collectives.

Per-core design (S=1024, H=16, Dh=64; TimelineSim cost model driven):
  * fp8e4(m3) DoubleRow matmuls (0.5 cyc/col) for the q/k/v projections
    (K_eff=256 chunk pairs over host-shipped xT/W in f8), and for the score
    matmuls G=q@r^T and AC=q@k^T (K=64 padded to DR pairs with a zeroed
    SBUF half).  Weights are host-scaled by 32 so f8 stays in normal range;
    scales cancel via a 32x DR identity (BD add), exp scale 0.125/1024 and
    Wo/32.
  * Relative shift: G evicted (x1/32) to f8 and round-tripped through a DRAM
    buffer of row length S+1 (col0=0); reading flat at offset S yields jax's
    _rel_shift exactly.  The BD add into the score PSUM is a DoubleRow
    identity matmul of the f8 bds tile (no extra exp / DVE pass).
  * One exp per score tile ([128,1024] f32 PSUM read across 2 banks) with
    accum_out denominators; reciprocals batched [128,8] per head; probs
    normalized in-place (split GpSimd/DVE).
  * prob^T via DMA XBAR transposes (14ns per 16x128 tile, on the DMA track)
    with a PE-transpose share for balance; PV with v stationary (N=512).
  * Output: avT(f16) @ (Wo/32)(f16) + residual on DVE; PSUM eviction load is
    split ACT/DVE; GpSimd handles memsets + most prob normalizations.
"""

import numpy as np
from contextlib import ExitStack

B = 8
D = 1024
H = 16
DH = 64
S = 1024

# routing knobs (tuned against TimelineSim)
G_EVICT_ACT_MOD = (2, 1)     # i%5<2 -> ACT else DVE
NORM_POOL_MOD = (3, 2)       # i%3<2 -> Pool else DVE
TRANSPOSE_PE_MOD = 3         # (2h+half)%MOD==0 -> PE transpose path, else DMA

_CACHED = {}


def _build():
    import concourse.bass as bass
    import concourse.bacc as bacc
    import concourse.tile as tile
    import concourse.mybir as mybir
    from concourse.ap import AP

    f32 = mybir.dt.float32
    f16 = mybir.dt.float16
    f8 = mybir.dt.float8e4
    EXP = mybir.ActivationFunctionType.Exp
    CPY = mybir.ActivationFunctionType.Copy
    DR = mybir.MatmulPerfMode.DoubleRow

    NB = S // 128            # 8 row blocks
    ESC = 0.125 / 1024.0     # exp scale: 1/sqrt(Dh) * 1/(32*32)

    nc = bacc.Bacc("TRN2", target_bir_lowering=False, debug=False)

    x8_d = nc.dram_tensor("x8", [D, S], f8, kind="ExternalInput")      # x^T, f8
    xr_d = nc.dram_tensor("xr", [S, D], f16, kind="ExternalInput")     # residual
    r8_d = nc.dram_tensor("r8", [D, S], f8, kind="ExternalInput")      # (32 pos@Wr)^T
    w8_d = nc.dram_tensor("w8", [D, 3 * D], f8, kind="ExternalInput")  # 32*Wqkv
    wo_d = nc.dram_tensor("wo", [D, D], f16, kind="ExternalInput")     # Wo/32
    i8_d = nc.dram_tensor("i8", [128, 256], f8, kind="ExternalInput")  # 128*I (DR, both halves)
    i16_d = nc.dram_tensor("i16", [128, 128], f16, kind="ExternalInput")
    out_d = nc.dram_tensor("out", [S, D], f32, kind="ExternalOutput")

    with tile.TileContext(nc) as tc, ExitStack() as es:
        p_x8 = es.enter_context(tc.tile_pool(name="x8", bufs=1))
        p_qk = es.enter_context(tc.tile_pool(name="qk", bufs=1))
        p_r8 = es.enter_context(tc.tile_pool(name="r8", bufs=1))
        p_v = es.enter_context(tc.tile_pool(name="v", bufs=1))
        p_avt = es.enter_context(tc.tile_pool(name="avt", bufs=1))
        p_wst = es.enter_context(tc.tile_pool(name="wst", bufs=2))
        p_i8 = es.enter_context(tc.tile_pool(name="i8", bufs=1))
        p_ga = es.enter_context(tc.tile_pool(name="ga", bufs=3))
        p_bds = es.enter_context(tc.tile_pool(name="bds", bufs=2))  # 2 per name x 2 names
        p_pu = es.enter_context(tc.tile_pool(name="pu", bufs=4))
        p_pt = es.enter_context(tc.tile_pool(name="pt", bufs=3))
        p_dn = es.enter_context(tc.tile_pool(name="dn", bufs=2))
        p_os = es.enter_context(tc.tile_pool(name="os", bufs=2))
        p_y = es.enter_context(tc.tile_pool(name="ydram", bufs=3, space="DRAM"))
        ps_s = es.enter_context(tc.tile_pool(name="pss", bufs=2, space="PSUM"))
        ps_g = es.enter_context(tc.tile_pool(name="psg", bufs=2, space="PSUM"))
        ps_av = es.enter_context(tc.tile_pool(name="psav", bufs=1, space="PSUM"))
        ps_t = es.enter_context(tc.tile_pool(name="pst", bufs=1, space="PSUM"))

        nev = [0]
        ndma = [0]

        def dq():
            ndma[0] += 1
            return nc.sync

        def evict_g(dst, src):
            """G eviction: f32 PSUM -> f8 (x 1/128; 128x identity restores), split ACT/DVE."""
            if nev[0] % G_EVICT_ACT_MOD[0] < G_EVICT_ACT_MOD[1]:
                nc.scalar.activation(dst, src, CPY, scale=1.0 / 128.0)
            else:
                nc.vector.tensor_scalar_mul(dst, src, 1.0 / 128.0)
            nev[0] += 1

        # ---- static loads ----
        x8s = p_x8.tile([128, 8, S], f8)
        nc.sync.dma_start(
            x8s[:], AP(x8_d[:].tensor, 0, [[S, 128], [128 * S, 8], [1, S]]))
        i8s = p_i8.tile([128, 256], f8)
        nc.sync.dma_start(i8s[:], i8_d[:])
        i16s = p_i8.tile([128, 128], f16)
        nc.sync.dma_start(i16s[:], i16_d[:])

        qk = [p_qk.tile([128, 2 * S], f8, name=f"qk{m}") for m in range(16)]
        for m in range(16):
            nc.gpsimd.memset(qk[m][:, S:2 * S], 0.0)
        r8s = [p_r8.tile([128, 2 * S], f8, name=f"r8{m}") for m in range(8)]
        for m in range(8):
            nc.sync.dma_start(r8s[m][:, 0:S], r8_d[m * 128:(m + 1) * 128, :])
            nc.gpsimd.memset(r8s[m][:, S:2 * S], 0.0)
        vsb = [p_v.tile([128, D], f16, name=f"v{m}") for m in range(NB)]
        avT = [p_avt.tile([128, S], f16, name=f"avT{m}") for m in range(8)]

        def wstream(col0):
            """One-DMA stage of W8 columns [col0, col0+512) as [128, 8, 512]."""
            w = p_wst.tile([128, 8, 512], f8)
            nc.sync.dma_start(
                w[:], AP(w8_d[:].tensor, col0,
                         [[3 * D, 128], [128 * 3 * D, 8], [1, 512]]))
            return w

        # ---- q,k projections (f8-out, DoubleRow, K_eff=256 chunks) ----
        for g in range(4):                       # 4 groups of 512 cols (q,q,k,k)
            w = wstream(512 * g)
            for b in range(4):                   # e-blocks within group
                m = 4 * g + b                    # qk tile index (0-7 q, 8-15 k)
                acc = ps_s.tile([128, S], f32, name="pss")
                for kc in range(4):
                    for bank in range(2):
                        nc.tensor.matmul(
                            acc[:, 512 * bank:512 * (bank + 1)],
                            w[:, 2 * kc:2 * kc + 2, 128 * b:128 * (b + 1)],
                            x8s[:, 2 * kc:2 * kc + 2, 512 * bank:512 * (bank + 1)],
                            start=(kc == 0), stop=(kc == 3), perf_mode=DR)
                nc.vector.tensor_copy(qk[m][:, 0:S], acc[:])

        # ---- v projection (f16-out, [s,e] orientation) ----
        for g in range(2):
            w = wstream(2 * D + 512 * g)
            for sb in range(NB):
                acc = ps_g.tile([128, 512], f32, name="pg")
                for kc in range(4):
                    nc.tensor.matmul(
                        acc[:],
                        x8s[:, 2 * kc:2 * kc + 2, 128 * sb:128 * (sb + 1)],
                        w[:, 2 * kc:2 * kc + 2, :],
                        start=(kc == 0), stop=(kc == 3), perf_mode=DR)
                nc.vector.tensor_copy(vsb[sb][:, 512 * g:512 * (g + 1)], acc[:])

        # ---- attention ----
        def dr2(t, lo, c0, n0, nn):
            """[64, 2, nn] DR operand view of a [128, 2S] tile: c=1 half zero."""
            return t[lo:lo + 64, :].rearrange(
                "p (c n) -> p c n", c=2)[:, :, c0 + n0:c0 + n0 + nn]

        def g_phase(h):
            qt, rt = qk[h // 2], r8s[h // 2]
            lo = 64 * (h % 2)
            y = p_y.tile([S * (S + 1)], f8, name=f"y{h % 3}")
            for half in range(2):
                ga = p_ga.tile([128, 4 * (S + 1)], f8)
                for b4 in range(4):
                    bi = 4 * half + b4
                    base = b4 * (S + 1)
                    nc.gpsimd.memset(ga[:, base:base + 1], 0.0)
                    for bank in range(2):
                        pg = ps_g.tile([128, 512], f32, name="pg")
                        nc.tensor.matmul(
                            pg[:], dr2(qt, lo, 0, 128 * bi, 128),
                            dr2(rt, lo, 0, 512 * bank, 512),
                            start=True, stop=True, perf_mode=DR)
                        evict_g(ga[:, base + 1 + 512 * bank:
                                    base + 1 + 512 * (bank + 1)], pg[:])
                dq().dma_start(
                    AP(y[:].tensor, half * 512 * (S + 1),
                       [[S + 1, 128], [128 * (S + 1), 4], [1, S + 1]]),
                    ga[:].rearrange("p (c n) -> p c n", c=4))
            return y

        def bds_fetch(h, y):
            lo = 64 * (h % 2)
            tiles = []
            for half in range(2):
                bds = p_bds.tile([128, 8, S], f8, name=f"bds{half}")
                dq().dma_start(
                    bds[lo:lo + 64, :, :],
                    AP(y[:].tensor, S + half * 512 * S,
                       [[S, 64], [64 * S, 8], [1, S]]))
                tiles.append(bds)
            return tiles

        def score_phase(h, bdss):
            qt, kt = qk[h // 2], qk[8 + h // 2]
            lo = 64 * (h % 2)
            den = p_dn.tile([128, 8], f32, name="den")
            rec = p_dn.tile([128, 8], f32, name="rec")
            pus = []
            for half in range(2):
                bds = bdss[half]
                pu = p_pu.tile([128, 4 * S], f16)
                pus.append(pu)
                for b4 in range(4):
                    ib = 4 * half + b4
                    pss = ps_s.tile([128, S], f32, name="pss")
                    for bank in range(2):
                        nc.tensor.matmul(
                            pss[:, 512 * bank:512 * (bank + 1)],
                            i8s[lo:lo + 64, :].rearrange(
                                "p (c n) -> p c n", c=2),
                            bds[lo:lo + 64, 2 * b4:2 * b4 + 2,
                                512 * bank:512 * (bank + 1)],
                            start=True, stop=False, perf_mode=DR)
                    for bank in range(2):
                        nc.tensor.matmul(
                            pss[:, 512 * bank:512 * (bank + 1)],
                            dr2(qt, lo, 0, 128 * ib, 128),
                            dr2(kt, lo, 0, 512 * bank, 512),
                            start=False, stop=True, perf_mode=DR)
                    nc.scalar.activation(
                        pu[:, S * b4:S * (b4 + 1)], pss[:], EXP, scale=ESC,
                        accum_out=den[:, ib:ib + 1])
            nc.vector.reciprocal(rec[:], den[:])

            for half in range(2):
                pu = pus[half]
                pt = p_pt.tile([128, 32, 128], f16)
                for b4 in range(4):
                    ib = 4 * half + b4
                    if (2 * ib + h) % NORM_POOL_MOD[0] < NORM_POOL_MOD[1]:
                        eng = nc.gpsimd
                    else:
                        eng = nc.vector
                    eng.tensor_scalar_mul(
                        pu[:, S * b4:S * (b4 + 1)], pu[:, S * b4:S * (b4 + 1)],
                        rec[:, ib:ib + 1])
                if TRANSPOSE_PE_MOD and (2 * h + half) % TRANSPOSE_PE_MOD == 0:
                    # PE transpose path: per ib, 8 [128,128] transposes -> psum
                    for b4 in range(4):
                        ptp = ps_t.tile([128, S], f16, name="ptp")
                        for jc in range(8):
                            nc.tensor.transpose(
                                ptp[:, 128 * jc:128 * (jc + 1)],
                                pu[:, S * b4 + 128 * jc:S * b4 + 128 * (jc + 1)],
                                i16s[:])
                        nc.vector.tensor_copy(
                            pt[:, 8 * b4:8 * (b4 + 1), :].rearrange(
                                "p a b -> p (a b)"),
                            ptp[:])
                else:
                    dq().dma_start_transpose(pt[:], pu[:])
                # PV: av[dh, (ib4, i)] accumulate over j chunks
                pav = ps_av.tile([64, 512], f32, name="pav")
                pt_r = pt[:].rearrange("p (b j) i -> p j b i", j=8)
                for jc in range(8):
                    nc.tensor.matmul(
                        pav[:], vsb[jc][:, DH * h:DH * (h + 1)],
                        pt_r[:, jc:jc + 1, :, :],
                        start=(jc == 0), stop=(jc == 7))
                nc.vector.tensor_copy(
                    avT[h // 2][lo:lo + 64, 512 * half:512 * (half + 1)], pav[:])

        ys = {0: g_phase(0), 1: g_phase(1)}
        bq = {0: bds_fetch(0, ys.pop(0))}
        for h in range(H):
            if h + 2 < H:
                ys[h + 2] = g_phase(h + 2)
            if h + 1 < H:
                bq[h + 1] = bds_fetch(h + 1, ys.pop(h + 1))
            score_phase(h, bq.pop(h))

        # ---- out = avT^T @ (Wo/32) + x ----
        wos = [p_r8.tile([128, D], f16, name=f"r8{m}") for m in range(8)]
        xrs = [p_qk.tile([128, D], f16, name=f"qk{m}") for m in range(8)]
        for m in range(8):
            nc.sync.dma_start(wos[m][:], wo_d[m * 128:(m + 1) * 128, :])
            nc.sync.dma_start(xrs[m][:], xr_d[m * 128:(m + 1) * 128, :])
        for ib in range(NB):
            osb = p_os.tile([128, D], f32)
            accs = [ps_g.tile([128, 512], f32, name="pg")
                    for k in range(2)]
            for kc in range(8):
                for bank in range(2):
                    nc.tensor.matmul(
                        accs[bank][:],
                        avT[kc][:, 128 * ib:128 * (ib + 1)],
                        wos[kc][:, 512 * bank:512 * (bank + 1)],
                        start=(kc == 0), stop=(kc == 7))
            for bank in range(2):
                nc.vector.tensor_add(
                    osb[:, 512 * bank:512 * (bank + 1)], accs[bank][:],
                    xrs[ib][:, 512 * bank:512 * (bank + 1)])
            dq().dma_start(out_d[ib * 128:(ib + 1) * 128, :], osb[:])

    nc.compile()
    return nc


def _pos_emb(S_, D_):
    pos_seq = np.arange(S_ - 1, -1, -1.0, dtype=np.float32)
    inv_freq = 1.0 / (10000.0 ** (np.arange(0, D_, 2.0, dtype=np.float32) / D_))
    sinusoid = np.einsum("i,j->ij", pos_seq, inv_freq).astype(np.float32)
    return np.concatenate([np.sin(sinusoid), np.cos(sinusoid)], axis=-1)


def _in_maps(x, Wqkv, Wr, Wo):
    import ml_dtypes
    f8 = ml_dtypes.float8_e4m3fn

    r = _pos_emb(S, D).astype(np.float32) @ np.asarray(Wr, dtype=np.float32)
    r8 = np.ascontiguousarray((32.0 * r.T).astype(f8)).view(np.uint8)
    w8 = np.ascontiguousarray(
        (32.0 * np.asarray(Wqkv, dtype=np.float32)).astype(f8)).view(np.uint8)
    wo = np.ascontiguousarray(
        (np.asarray(Wo, dtype=np.float32) / 32.0).astype(np.float16))
    i8 = np.zeros((128, 2, 128), dtype=f8)
    for p in range(64):
        for c in range(2):
            i8[p, c, p + 64 * c] = 128.0
            i8[64 + p, c, p + 64 * c] = 128.0
    i8 = np.ascontiguousarray(i8.reshape(128, 256)).view(np.uint8)
    i16 = np.eye(128, dtype=np.float16)

    maps = []
    for b in range(B):
        xb = np.asarray(x[b], dtype=np.float32)
        maps.append({
            "x8": np.ascontiguousarray(xb.T.astype(f8)).view(np.uint8),
            "xr": np.ascontiguousarray(xb.astype(np.float16)),
            "r8": r8, "w8": w8, "wo": wo, "i8": i8, "i16": i16,
        })
    return maps


def kernel(inputs, mask, Wqkv, Wr, Wo):
    from concourse.bass_utils import run_bass_kernel_spmd

    if "nc" not in _CACHED:
        _CACHED["nc"] = _build()
    nc = _CACHED["nc"]
    maps = _in_maps(np.asarray(inputs, dtype=np.float32), Wqkv, Wr, Wo)
    res = run_bass_kernel_spmd(nc, maps, core_ids=list(range(B)))
    out = np.stack([res.results[b]["out"] for b in range(B)], axis=0)
    return out.astype(np.float32)


# revision 30
# speedup vs baseline: 1.0154x; 1.0154x over previous
"""Trainium2 Bass kernel for MultiHeadRelativeSelfAttention (Transformer-XL).

Sharding: data-parallel over batch; 8 cores x 1 batch element, no collectives.

Per-core design (S=1024, H=16, Dh=64; TimelineSim cost model driven):
  * fp8e4(m3) DoubleRow matmuls (0.5 cyc/col) for the q/k/v projections
    (K_eff=256 chunk pairs over host-shipped xT/W in f8), and for the score
    matmuls G=q@r^T and AC=q@k^T (K=64 padded to DR pairs with a zeroed
    SBUF half).  Weights are host-scaled by 32 so f8 stays in normal range;
    scales cancel via a 32x DR identity (BD add), exp scale 0.125/1024 and
    Wo/32.
  * Relative shift: G evicted (x1/32) to f8 and round-tripped through a DRAM
    buffer of row length S+1 (col0=0); reading flat at offset S yields jax's
    _rel_shift exactly.  The BD add into the score PSUM is a DoubleRow
    identity matmul of the f8 bds tile (no extra exp / DVE pass).
  * One exp per score tile ([128,1024] f32 PSUM read across 2 banks) with
    accum_out denominators; reciprocals batched [128,8] per head; probs
    normalized in-place (split GpSimd/DVE).
  * prob^T via DMA XBAR transposes (14ns per 16x128 tile, on the DMA track)
    with a PE-transpose share for balance; PV with v stationary (N=512).
  * Output: avT(f16) @ (Wo/32)(f16) + residual on DVE; PSUM eviction load is
    split ACT/DVE; GpSimd handles memsets + most prob normalizations.
"""

import numpy as np
from contextlib import ExitStack

B = 8
D = 1024
H = 16
DH = 64
S = 1024

# routing knobs (tuned against TimelineSim)
G_EVICT_ACT_MOD = (2, 1)     # i%5<2 -> ACT else DVE
NORM_POOL_MOD = (3, 2)       # i%3<2 -> Pool else DVE
TRANSPOSE_PE_MOD = 3         # (2h+half)%MOD==0 -> PE transpose path, else DMA

_CACHED = {}


def _build():
    import concourse.bass as bass
    import concourse.bacc as bacc
    import concourse.tile as tile
    import concourse.mybir as mybir
    from concourse.ap import AP

    f32 = mybir.dt.float32
    f16 = mybir.dt.float16
    f8 = mybir.dt.float8e4
    EXP = mybir.ActivationFunctionType.Exp
    CPY = mybir.ActivationFunctionType.Copy
    DR = mybir.MatmulPerfMode.DoubleRow

    NB = S // 128            # 8 row blocks
    ESC = 0.125 / 1024.0     # exp scale: 1/sqrt(Dh) * 1/(32*32)

    nc = bacc.Bacc("TRN2", target_bir_lowering=False, debug=False)

    x8_d = nc.dram_tensor("x8", [D, S], f8, kind="ExternalInput")      # x^T, f8
    xr_d = nc.dram_tensor("xr", [S, D], f16, kind="ExternalInput")     # residual
    r8_d = nc.dram_tensor("r8", [D, S], f8, kind="ExternalInput")      # (32 pos@Wr)^T
    w8_d = nc.dram_tensor("w8", [D, 3 * D], f8, kind="ExternalInput")  # 32*Wqkv
    wo_d = nc.dram_tensor("wo", [D, D], f8, kind="ExternalInput")      # 32*Wo
    i8_d = nc.dram_tensor("i8", [128, 256], f8, kind="ExternalInput")  # 128*I (DR, both halves)
    i16_d = nc.dram_tensor("i16", [128, 128], f16, kind="ExternalInput")
    out_d = nc.dram_tensor("out", [S, D], f32, kind="ExternalOutput")

    with tile.TileContext(nc) as tc, ExitStack() as es:
        p_x8 = es.enter_context(tc.tile_pool(name="x8", bufs=1))
        p_qk = es.enter_context(tc.tile_pool(name="qk", bufs=1))
        p_r8 = es.enter_context(tc.tile_pool(name="r8", bufs=1))
        p_v = es.enter_context(tc.tile_pool(name="v", bufs=1))
        p_avt = es.enter_context(tc.tile_pool(name="avt", bufs=1))
        p_wst = es.enter_context(tc.tile_pool(name="wst", bufs=2))
        p_i8 = es.enter_context(tc.tile_pool(name="i8", bufs=1))
        p_ga = es.enter_context(tc.tile_pool(name="ga", bufs=3))
        p_bds = es.enter_context(tc.tile_pool(name="bds", bufs=2))  # 2 per name x 2 names
        p_pu = es.enter_context(tc.tile_pool(name="pu", bufs=4))
        p_pt = es.enter_context(tc.tile_pool(name="pt", bufs=3))
        p_dn = es.enter_context(tc.tile_pool(name="dn", bufs=2))
        p_os = es.enter_context(tc.tile_pool(name="os", bufs=2))
        p_y = es.enter_context(tc.tile_pool(name="ydram", bufs=3, space="DRAM"))
        ps_s = es.enter_context(tc.tile_pool(name="pss", bufs=2, space="PSUM"))
        ps_g = es.enter_context(tc.tile_pool(name="psg", bufs=2, space="PSUM"))
        ps_av = es.enter_context(tc.tile_pool(name="psav", bufs=1, space="PSUM"))
        ps_t = es.enter_context(tc.tile_pool(name="pst", bufs=1, space="PSUM"))

        nev = [0]
        ndma = [0]

        def dq():
            ndma[0] += 1
            return nc.sync

        def evict_g(dst, src):
            """G eviction: f32 PSUM -> f8 (x 1/128; 128x identity restores), split ACT/DVE."""
            if nev[0] % G_EVICT_ACT_MOD[0] < G_EVICT_ACT_MOD[1]:
                nc.scalar.activation(dst, src, CPY, scale=1.0 / 128.0)
            else:
                nc.vector.tensor_scalar_mul(dst, src, 1.0 / 128.0)
            nev[0] += 1

        # ---- static loads ----
        x8s = p_x8.tile([128, 8, S], f8)
        nc.sync.dma_start(
            x8s[:], AP(x8_d[:].tensor, 0, [[S, 128], [128 * S, 8], [1, S]]))
        i8s = p_i8.tile([128, 256], f8)
        nc.sync.dma_start(i8s[:], i8_d[:])
        i16s = p_i8.tile([128, 128], f16)
        nc.sync.dma_start(i16s[:], i16_d[:])

        qk = [p_qk.tile([128, 2 * S], f8, name=f"qk{m}") for m in range(16)]
        for m in range(16):
            nc.gpsimd.memset(qk[m][:, S:2 * S], 0.0)
        r8s = [p_r8.tile([128, 2 * S], f8, name=f"r8{m}") for m in range(8)]
        for m in range(8):
            nc.sync.dma_start(r8s[m][:, 0:S], r8_d[m * 128:(m + 1) * 128, :])
            nc.gpsimd.memset(r8s[m][:, S:2 * S], 0.0)
        vsb = [p_v.tile([128, D], f16, name=f"v{m}") for m in range(NB)]
        avTb = p_avt.tile([128, 8 * S], f8)

        def wstream(col0):
            """One-DMA stage of W8 columns [col0, col0+512) as [128, 8, 512]."""
            w = p_wst.tile([128, 8, 512], f8)
            nc.sync.dma_start(
                w[:], AP(w8_d[:].tensor, col0,
                         [[3 * D, 128], [128 * 3 * D, 8], [1, 512]]))
            return w

        # ---- q,k projections (f8-out, DoubleRow, K_eff=256 chunks) ----
        for g in range(4):                       # 4 groups of 512 cols (q,q,k,k)
            w = wstream(512 * g)
            for b in range(4):                   # e-blocks within group
                m = 4 * g + b                    # qk tile index (0-7 q, 8-15 k)
                acc = ps_s.tile([128, S], f32, name="pss")
                for kc in range(4):
                    for bank in range(2):
                        nc.tensor.matmul(
                            acc[:, 512 * bank:512 * (bank + 1)],
                            w[:, 2 * kc:2 * kc + 2, 128 * b:128 * (b + 1)],
                            x8s[:, 2 * kc:2 * kc + 2, 512 * bank:512 * (bank + 1)],
                            start=(kc == 0), stop=(kc == 3), perf_mode=DR)
                nc.vector.tensor_copy(qk[m][:, 0:S], acc[:])

        # ---- v projection (f16-out, [s,e] orientation) ----
        for g in range(2):
            w = wstream(2 * D + 512 * g)
            for sb in range(NB):
                acc = ps_g.tile([128, 512], f32, name="pg")
                for kc in range(4):
                    nc.tensor.matmul(
                        acc[:],
                        x8s[:, 2 * kc:2 * kc + 2, 128 * sb:128 * (sb + 1)],
                        w[:, 2 * kc:2 * kc + 2, :],
                        start=(kc == 0), stop=(kc == 3), perf_mode=DR)
                nc.vector.tensor_copy(vsb[sb][:, 512 * g:512 * (g + 1)], acc[:])

        # ---- attention ----
        def dr2(t, lo, c0, n0, nn):
            """[64, 2, nn] DR operand view of a [128, 2S] tile: c=1 half zero."""
            return t[lo:lo + 64, :].rearrange(
                "p (c n) -> p c n", c=2)[:, :, c0 + n0:c0 + n0 + nn]

        def g_phase(h):
            qt, rt = qk[h // 2], r8s[h // 2]
            lo = 64 * (h % 2)
            y = p_y.tile([S * (S + 1)], f8, name=f"y{h % 3}")
            for half in range(2):
                ga = p_ga.tile([128, 4 * (S + 1)], f8)
                for b4 in range(4):
                    bi = 4 * half + b4
                    base = b4 * (S + 1)
                    nc.gpsimd.memset(ga[:, base:base + 1], 0.0)
                    for bank in range(2):
                        pg = ps_g.tile([128, 512], f32, name="pg")
                        nc.tensor.matmul(
                            pg[:], dr2(qt, lo, 0, 128 * bi, 128),
                            dr2(rt, lo, 0, 512 * bank, 512),
                            start=True, stop=True, perf_mode=DR)
                        evict_g(ga[:, base + 1 + 512 * bank:
                                    base + 1 + 512 * (bank + 1)], pg[:])
                dq().dma_start(
                    AP(y[:].tensor, half * 512 * (S + 1),
                       [[S + 1, 128], [128 * (S + 1), 4], [1, S + 1]]),
                    ga[:].rearrange("p (c n) -> p c n", c=4))
            return y

        def bds_fetch(h, y):
            lo = 64 * (h % 2)
            tiles = []
            for half in range(2):
                bds = p_bds.tile([128, 8, S], f8, name=f"bds{half}")
                dq().dma_start(
                    bds[lo:lo + 64, :, :],
                    AP(y[:].tensor, S + half * 512 * S,
                       [[S, 64], [64 * S, 8], [1, S]]))
                tiles.append(bds)
            return tiles

        def score_phase(h, bdss):
            qt, kt = qk[h // 2], qk[8 + h // 2]
            lo = 64 * (h % 2)
            den = p_dn.tile([128, 8], f32, name="den")
            rec = p_dn.tile([128, 8], f32, name="rec")
            pus = []
            for half in range(2):
                bds = bdss[half]
                pu = p_pu.tile([128, 4 * S], f16)
                pus.append(pu)
                for b4 in range(4):
                    ib = 4 * half + b4
                    pss = ps_s.tile([128, S], f32, name="pss")
                    for bank in range(2):
                        nc.tensor.matmul(
                            pss[:, 512 * bank:512 * (bank + 1)],
                            i8s[lo:lo + 64, :].rearrange(
                                "p (c n) -> p c n", c=2),
                            bds[lo:lo + 64, 2 * b4:2 * b4 + 2,
                                512 * bank:512 * (bank + 1)],
                            start=True, stop=False, perf_mode=DR)
                    for bank in range(2):
                        nc.tensor.matmul(
                            pss[:, 512 * bank:512 * (bank + 1)],
                            dr2(qt, lo, 0, 128 * ib, 128),
                            dr2(kt, lo, 0, 512 * bank, 512),
                            start=False, stop=True, perf_mode=DR)
                    nc.scalar.activation(
                        pu[:, S * b4:S * (b4 + 1)], pss[:], EXP, scale=ESC,
                        accum_out=den[:, ib:ib + 1])
            nc.vector.reciprocal(rec[:], den[:])

            for half in range(2):
                pu = pus[half]
                pt = p_pt.tile([128, 32, 128], f16)
                for b4 in range(4):
                    ib = 4 * half + b4
                    if (2 * ib + h) % NORM_POOL_MOD[0] < NORM_POOL_MOD[1]:
                        eng = nc.gpsimd
                    else:
                        eng = nc.vector
                    eng.tensor_scalar_mul(
                        pu[:, S * b4:S * (b4 + 1)], pu[:, S * b4:S * (b4 + 1)],
                        rec[:, ib:ib + 1])
                if TRANSPOSE_PE_MOD and (2 * h + half) % TRANSPOSE_PE_MOD == 0:
                    # PE transpose path: per ib, 8 [128,128] transposes -> psum
                    for b4 in range(4):
                        ptp = ps_t.tile([128, S], f16, name="ptp")
                        for jc in range(8):
                            nc.tensor.transpose(
                                ptp[:, 128 * jc:128 * (jc + 1)],
                                pu[:, S * b4 + 128 * jc:S * b4 + 128 * (jc + 1)],
                                i16s[:])
                        nc.vector.tensor_copy(
                            pt[:, 8 * b4:8 * (b4 + 1), :].rearrange(
                                "p a b -> p (a b)"),
                            ptp[:])
                else:
                    dq().dma_start_transpose(pt[:], pu[:])
                # PV: av[dh, (ib4, i)] accumulate over j chunks
                pav = ps_av.tile([64, 512], f32, name="pav")
                pt_r = pt[:].rearrange("p (b j) i -> p j b i", j=8)
                for jc in range(8):
                    nc.tensor.matmul(
                        pav[:], vsb[jc][:, DH * h:DH * (h + 1)],
                        pt_r[:, jc:jc + 1, :, :],
                        start=(jc == 0), stop=(jc == 7))
                nc.vector.tensor_copy(
                    avTb[lo:lo + 64, (h // 2) * S + 512 * half:
                         (h // 2) * S + 512 * (half + 1)], pav[:])

        ys = {0: g_phase(0), 1: g_phase(1)}
        bq = {0: bds_fetch(0, ys.pop(0))}
        for h in range(H):
            if h + 2 < H:
                ys[h + 2] = g_phase(h + 2)
            if h + 1 < H:
                bq[h + 1] = bds_fetch(h + 1, ys.pop(h + 1))
            score_phase(h, bq.pop(h))

        # ---- out = (32 avT)^T @ (32 Wo) / 1024 + x  (DoubleRow f8) ----
        wo8 = []
        for t in range(4):                      # (bank, kc-half) quarter tiles
            bank, kh = t // 2, t % 2
            wt = p_r8.tile([128, 4, 512], f8, name=f"r8{t}")
            nc.sync.dma_start(
                wt[:], AP(wo_d[:].tensor, 512 * kh * D + 512 * bank,
                          [[D, 128], [128 * D, 4], [1, 512]]))
            wo8.append(wt)
        xrs = [p_qk.tile([128, D], f16, name=f"qk{m}") for m in range(8)]
        for m in range(8):
            nc.sync.dma_start(xrs[m][:], xr_d[m * 128:(m + 1) * 128, :])
        avTr = avTb[:].rearrange("p (c n) -> p c n", c=8)
        for ib in range(NB):
            osb = p_os.tile([128, D], f32)
            accs = [ps_g.tile([128, 512], f32, name="pg")
                    for k in range(2)]
            for kc in range(4):
                for bank in range(2):
                    nc.tensor.matmul(
                        accs[bank][:],
                        avTr[:, 2 * kc:2 * kc + 2, 128 * ib:128 * (ib + 1)],
                        wo8[2 * bank + kc // 2][:, 2 * (kc % 2):
                                                2 * (kc % 2) + 2, :],
                        start=(kc == 0), stop=(kc == 3), perf_mode=DR)
            for bank in range(2):
                sl = slice(512 * bank, 512 * (bank + 1))
                nc.vector.tensor_scalar_mul(osb[:, sl], accs[bank][:],
                                            1.0 / 1024.0)
                nc.vector.tensor_add(osb[:, sl], osb[:, sl], xrs[ib][:, sl])
            dq().dma_start(out_d[ib * 128:(ib + 1) * 128, :], osb[:])

    nc.compile()
    return nc


def _pos_emb(S_, D_):
    pos_seq = np.arange(S_ - 1, -1, -1.0, dtype=np.float32)
    inv_freq = 1.0 / (10000.0 ** (np.arange(0, D_, 2.0, dtype=np.float32) / D_))
    sinusoid = np.einsum("i,j->ij", pos_seq, inv_freq).astype(np.float32)
    return np.concatenate([np.sin(sinusoid), np.cos(sinusoid)], axis=-1)


def _in_maps(x, Wqkv, Wr, Wo):
    import ml_dtypes
    f8 = ml_dtypes.float8_e4m3fn

    r = _pos_emb(S, D).astype(np.float32) @ np.asarray(Wr, dtype=np.float32)
    r8 = np.ascontiguousarray((32.0 * r.T).astype(f8)).view(np.uint8)
    w8 = np.ascontiguousarray(
        (32.0 * np.asarray(Wqkv, dtype=np.float32)).astype(f8)).view(np.uint8)
    wo = np.ascontiguousarray(
        (32.0 * np.asarray(Wo, dtype=np.float32)).astype(f8)).view(np.uint8)
    i8 = np.zeros((128, 2, 128), dtype=f8)
    for p in range(64):
        for c in range(2):
            i8[p, c, p + 64 * c] = 128.0
            i8[64 + p, c, p + 64 * c] = 128.0
    i8 = np.ascontiguousarray(i8.reshape(128, 256)).view(np.uint8)
    i16 = np.eye(128, dtype=np.float16)

    maps = []
    for b in range(B):
        xb = np.asarray(x[b], dtype=np.float32)
        maps.append({
            "x8": np.ascontiguousarray(xb.T.astype(f8)).view(np.uint8),
            "xr": np.ascontiguousarray(xb.astype(np.float16)),
            "r8": r8, "w8": w8, "wo": wo, "i8": i8, "i16": i16,
        })
    return maps


def kernel(inputs, mask, Wqkv, Wr, Wo):
    from concourse.bass_utils import run_bass_kernel_spmd

    if "nc" not in _CACHED:
        _CACHED["nc"] = _build()
    nc = _CACHED["nc"]
    maps = _in_maps(np.asarray(inputs, dtype=np.float32), Wqkv, Wr, Wo)
    res = run_bass_kernel_spmd(nc, maps, core_ids=list(range(B)))
    out = np.stack([res.results[b]["out"] for b in range(B)], axis=0)
    return out.astype(np.float32)


# revision 33
# speedup vs baseline: 1.0163x; 1.0009x over previous
"""Trainium2 Bass kernel for MultiHeadRelativeSelfAttention (Transformer-XL).

Sharding: data-parallel over batch; 8 cores x 1 batch element, no collectives.

Per-core design (S=1024, H=16, Dh=64; TimelineSim cost model driven):
  * fp8e4(m3) DoubleRow matmuls (0.5 cyc/col) for the q/k/v projections
    (K_eff=256 chunk pairs over host-shipped xT/W in f8), and for the score
    matmuls G=q@r^T and AC=q@k^T (K=64 padded to DR pairs with a zeroed
    SBUF half).  Weights are host-scaled by 32 so f8 stays in normal range;
    scales cancel via a 32x DR identity (BD add), exp scale 0.125/1024 and
    Wo/32.
  * Relative shift: G evicted (x1/32) to f8 and round-tripped through a DRAM
    buffer of row length S+1 (col0=0); reading flat at offset S yields jax's
    _rel_shift exactly.  The BD add into the score PSUM is a DoubleRow
    identity matmul of the f8 bds tile (no extra exp / DVE pass).
  * One exp per score tile ([128,1024] f32 PSUM read across 2 banks) with
    accum_out denominators; reciprocals batched [128,8] per head; probs
    normalized in-place (split GpSimd/DVE).
  * prob^T via DMA XBAR transposes (14ns per 16x128 tile, on the DMA track)
    with a PE-transpose share for balance; PV with v stationary (N=512).
  * Output: avT(f16) @ (Wo/32)(f16) + residual on DVE; PSUM eviction load is
    split ACT/DVE; GpSimd handles memsets + most prob normalizations.
"""

import numpy as np
from contextlib import ExitStack

B = 8
D = 1024
H = 16
DH = 64
S = 1024

# routing knobs (tuned against TimelineSim)
G_EVICT_ACT_MOD = (2, 1)     # i%5<2 -> ACT else DVE
NORM_POOL_MOD = (3, 2)       # i%3<2 -> Pool else DVE
TRANSPOSE_PE_MOD = 3         # (2h+half)%MOD==0 -> PE transpose path, else DMA

_CACHED = {}


def _build():
    import concourse.bass as bass
    import concourse.bacc as bacc
    import concourse.tile as tile
    import concourse.mybir as mybir
    from concourse.ap import AP

    f32 = mybir.dt.float32
    f16 = mybir.dt.float16
    f8 = mybir.dt.float8e4
    EXP = mybir.ActivationFunctionType.Exp
    CPY = mybir.ActivationFunctionType.Copy
    DR = mybir.MatmulPerfMode.DoubleRow

    NB = S // 128            # 8 row blocks
    ESC = 0.125 / 1024.0     # exp scale: 1/sqrt(Dh) * 1/(32*32)

    nc = bacc.Bacc("TRN2", target_bir_lowering=False, debug=False)

    x8_d = nc.dram_tensor("x8", [D, S], f8, kind="ExternalInput")      # x^T, f8
    xr_d = nc.dram_tensor("xr", [S, D], f16, kind="ExternalInput")     # residual
    r8_d = nc.dram_tensor("r8", [D, S], f8, kind="ExternalInput")      # (32 pos@Wr)^T
    w8_d = nc.dram_tensor("w8", [D, 3 * D], f8, kind="ExternalInput")  # 32*Wqkv
    wo_d = nc.dram_tensor("wo", [D, D], f8, kind="ExternalInput")      # 32*Wo
    i8_d = nc.dram_tensor("i8", [128, 256], f8, kind="ExternalInput")  # 128*I (DR, both halves)
    i16_d = nc.dram_tensor("i16", [128, 128], f16, kind="ExternalInput")
    out_d = nc.dram_tensor("out", [S, D], f32, kind="ExternalOutput")

    with tile.TileContext(nc) as tc, ExitStack() as es:
        p_x8 = es.enter_context(tc.tile_pool(name="x8", bufs=1))
        p_qk = es.enter_context(tc.tile_pool(name="qk", bufs=1))
        p_r8 = es.enter_context(tc.tile_pool(name="r8", bufs=1))
        p_v = es.enter_context(tc.tile_pool(name="v", bufs=1))
        p_avt = es.enter_context(tc.tile_pool(name="avt", bufs=1))
        p_wst = es.enter_context(tc.tile_pool(name="wst", bufs=2))
        p_i8 = es.enter_context(tc.tile_pool(name="i8", bufs=1))
        p_ga = es.enter_context(tc.tile_pool(name="ga", bufs=3))
        p_bds = es.enter_context(tc.tile_pool(name="bds", bufs=2))  # 2 per name x 2 names
        p_pu = es.enter_context(tc.tile_pool(name="pu", bufs=5))
        p_pt = es.enter_context(tc.tile_pool(name="pt", bufs=3))
        p_dn = es.enter_context(tc.tile_pool(name="dn", bufs=2))
        p_os = es.enter_context(tc.tile_pool(name="os", bufs=2))
        p_y = es.enter_context(tc.tile_pool(name="ydram", bufs=3, space="DRAM"))
        ps_s = es.enter_context(tc.tile_pool(name="pss", bufs=2, space="PSUM"))
        ps_g = es.enter_context(tc.tile_pool(name="psg", bufs=2, space="PSUM"))
        ps_av = es.enter_context(tc.tile_pool(name="psav", bufs=1, space="PSUM"))
        ps_t = es.enter_context(tc.tile_pool(name="pst", bufs=1, space="PSUM"))

        nev = [0]
        ndma = [0]

        def dq():
            ndma[0] += 1
            return nc.sync

        def evict_g(dst, src):
            """G eviction: f32 PSUM -> f8 (x 1/128; 128x identity restores), split ACT/DVE."""
            if nev[0] % G_EVICT_ACT_MOD[0] < G_EVICT_ACT_MOD[1]:
                nc.scalar.activation(dst, src, CPY, scale=1.0 / 128.0)
            else:
                nc.vector.tensor_scalar_mul(dst, src, 1.0 / 128.0)
            nev[0] += 1

        # ---- static loads ----
        x8s = p_x8.tile([128, 8, S], f8)
        nc.sync.dma_start(
            x8s[:], AP(x8_d[:].tensor, 0, [[S, 128], [128 * S, 8], [1, S]]))
        i8s = p_i8.tile([128, 256], f8)
        nc.sync.dma_start(i8s[:], i8_d[:])
        i16s = p_i8.tile([128, 128], f16)
        nc.sync.dma_start(i16s[:], i16_d[:])

        qk = [p_qk.tile([128, 2 * S], f8, name=f"qk{m}") for m in range(16)]
        for m in range(16):
            nc.gpsimd.memset(qk[m][:, S:2 * S], 0.0)
        r8s = [p_r8.tile([128, 2 * S], f8, name=f"r8{m}") for m in range(8)]
        for m in range(8):
            nc.sync.dma_start(r8s[m][:, 0:S], r8_d[m * 128:(m + 1) * 128, :])
            nc.gpsimd.memset(r8s[m][:, S:2 * S], 0.0)
        vsb = [p_v.tile([128, D], f16, name=f"v{m}") for m in range(NB)]
        avTb = p_avt.tile([128, 8 * S], f8)

        def wstream(col0):
            """One-DMA stage of W8 columns [col0, col0+512) as [128, 8, 512]."""
            w = p_wst.tile([128, 8, 512], f8)
            nc.sync.dma_start(
                w[:], AP(w8_d[:].tensor, col0,
                         [[3 * D, 128], [128 * 3 * D, 8], [1, 512]]))
            return w

        # ---- q,k projections (f8-out, DoubleRow, K_eff=256 chunks) ----
        for g in range(4):                       # 4 groups of 512 cols (q,q,k,k)
            w = wstream(512 * g)
            for b in range(4):                   # e-blocks within group
                m = 4 * g + b                    # qk tile index (0-7 q, 8-15 k)
                acc = ps_s.tile([128, S], f32, name="pss")
                for kc in range(4):
                    for bank in range(2):
                        nc.tensor.matmul(
                            acc[:, 512 * bank:512 * (bank + 1)],
                            w[:, 2 * kc:2 * kc + 2, 128 * b:128 * (b + 1)],
                            x8s[:, 2 * kc:2 * kc + 2, 512 * bank:512 * (bank + 1)],
                            start=(kc == 0), stop=(kc == 3), perf_mode=DR)
                nc.vector.tensor_copy(qk[m][:, 0:S], acc[:])

        # ---- v projection (f16-out, [s,e] orientation) ----
        for g in range(2):
            w = wstream(2 * D + 512 * g)
            for sb in range(NB):
                acc = ps_g.tile([128, 512], f32, name="pg")
                for kc in range(4):
                    nc.tensor.matmul(
                        acc[:],
                        x8s[:, 2 * kc:2 * kc + 2, 128 * sb:128 * (sb + 1)],
                        w[:, 2 * kc:2 * kc + 2, :],
                        start=(kc == 0), stop=(kc == 3), perf_mode=DR)
                nc.vector.tensor_copy(vsb[sb][:, 512 * g:512 * (g + 1)], acc[:])

        # ---- attention ----
        def dr2(t, lo, c0, n0, nn):
            """[64, 2, nn] DR operand view of a [128, 2S] tile: c=1 half zero."""
            return t[lo:lo + 64, :].rearrange(
                "p (c n) -> p c n", c=2)[:, :, c0 + n0:c0 + n0 + nn]

        def g_phase(h):
            qt, rt = qk[h // 2], r8s[h // 2]
            lo = 64 * (h % 2)
            y = p_y.tile([S * (S + 1)], f8, name=f"y{h % 3}")
            for half in range(2):
                ga = p_ga.tile([128, 4 * (S + 1)], f8)
                for b4 in range(4):
                    bi = 4 * half + b4
                    base = b4 * (S + 1)
                    nc.gpsimd.memset(ga[:, base:base + 1], 0.0)
                    for bank in range(2):
                        pg = ps_g.tile([128, 512], f32, name="pg")
                        nc.tensor.matmul(
                            pg[:], dr2(qt, lo, 0, 128 * bi, 128),
                            dr2(rt, lo, 0, 512 * bank, 512),
                            start=True, stop=True, perf_mode=DR)
                        evict_g(ga[:, base + 1 + 512 * bank:
                                    base + 1 + 512 * (bank + 1)], pg[:])
                dq().dma_start(
                    AP(y[:].tensor, half * 512 * (S + 1),
                       [[S + 1, 128], [128 * (S + 1), 4], [1, S + 1]]),
                    ga[:].rearrange("p (c n) -> p c n", c=4))
            return y

        def bds_fetch(h, y):
            lo = 64 * (h % 2)
            tiles = []
            for half in range(2):
                bds = p_bds.tile([128, 8, S], f8, name=f"bds{half}")
                dq().dma_start(
                    bds[lo:lo + 64, :, :],
                    AP(y[:].tensor, S + half * 512 * S,
                       [[S, 64], [64 * S, 8], [1, S]]))
                tiles.append(bds)
            return tiles

        def score_phase(h, bdss):
            qt, kt = qk[h // 2], qk[8 + h // 2]
            lo = 64 * (h % 2)
            den = p_dn.tile([128, 8], f32, name="den")
            rec = p_dn.tile([128, 8], f32, name="rec")
            pus = []
            for half in range(2):
                bds = bdss[half]
                pu = p_pu.tile([128, 4 * S], f16)
                pus.append(pu)
                for b4 in range(4):
                    ib = 4 * half + b4
                    pss = ps_s.tile([128, S], f32, name="pss")
                    for bank in range(2):
                        nc.tensor.matmul(
                            pss[:, 512 * bank:512 * (bank + 1)],
                            i8s[lo:lo + 64, :].rearrange(
                                "p (c n) -> p c n", c=2),
                            bds[lo:lo + 64, 2 * b4:2 * b4 + 2,
                                512 * bank:512 * (bank + 1)],
                            start=True, stop=False, perf_mode=DR)
                    for bank in range(2):
                        nc.tensor.matmul(
                            pss[:, 512 * bank:512 * (bank + 1)],
                            dr2(qt, lo, 0, 128 * ib, 128),
                            dr2(kt, lo, 0, 512 * bank, 512),
                            start=False, stop=True, perf_mode=DR)
                    nc.scalar.activation(
                        pu[:, S * b4:S * (b4 + 1)], pss[:], EXP, scale=ESC,
                        accum_out=den[:, ib:ib + 1])
            nc.vector.reciprocal(rec[:], den[:])

            for half in range(2):
                pu = pus[half]
                pt = p_pt.tile([128, 32, 128], f16)
                for b4 in range(4):
                    ib = 4 * half + b4
                    if (2 * ib + h) % NORM_POOL_MOD[0] < NORM_POOL_MOD[1]:
                        eng = nc.gpsimd
                    else:
                        eng = nc.vector
                    eng.tensor_scalar_mul(
                        pu[:, S * b4:S * (b4 + 1)], pu[:, S * b4:S * (b4 + 1)],
                        rec[:, ib:ib + 1])
                if TRANSPOSE_PE_MOD and (2 * h + half) % TRANSPOSE_PE_MOD == 0:
                    # PE transpose path: per ib, 8 [128,128] transposes -> psum
                    for b4 in range(4):
                        ptp = ps_t.tile([128, S], f16, name="ptp")
                        for jc in range(8):
                            nc.tensor.transpose(
                                ptp[:, 128 * jc:128 * (jc + 1)],
                                pu[:, S * b4 + 128 * jc:S * b4 + 128 * (jc + 1)],
                                i16s[:])
                        nc.vector.tensor_copy(
                            pt[:, 8 * b4:8 * (b4 + 1), :].rearrange(
                                "p a b -> p (a b)"),
                            ptp[:])
                else:
                    dq().dma_start_transpose(pt[:], pu[:])
                # PV: av[dh, (ib4, i)] accumulate over j chunks
                pav = ps_av.tile([64, 512], f32, name="pav")
                pt_r = pt[:].rearrange("p (b j) i -> p j b i", j=8)
                for jc in range(8):
                    nc.tensor.matmul(
                        pav[:], vsb[jc][:, DH * h:DH * (h + 1)],
                        pt_r[:, jc:jc + 1, :, :],
                        start=(jc == 0), stop=(jc == 7))
                nc.vector.tensor_copy(
                    avTb[lo:lo + 64, (h // 2) * S + 512 * half:
                         (h // 2) * S + 512 * (half + 1)], pav[:])

        ys = {0: g_phase(0), 1: g_phase(1)}
        bq = {0: bds_fetch(0, ys.pop(0))}
        for h in range(H):
            if h + 2 < H:
                ys[h + 2] = g_phase(h + 2)
            if h + 1 < H:
                bq[h + 1] = bds_fetch(h + 1, ys.pop(h + 1))
            score_phase(h, bq.pop(h))

        # ---- out = (32 avT)^T @ (32 Wo) / 1024 + x  (DoubleRow f8) ----
        wo8 = []
        for t in range(4):                      # (bank, kc-half) quarter tiles
            bank, kh = t // 2, t % 2
            wt = p_r8.tile([128, 4, 512], f8, name=f"r8{t}")
            nc.sync.dma_start(
                wt[:], AP(wo_d[:].tensor, 512 * kh * D + 512 * bank,
                          [[D, 128], [128 * D, 4], [1, 512]]))
            wo8.append(wt)
        xrs = [p_qk.tile([128, D], f16, name=f"qk{m}") for m in range(8)]
        for m in range(8):
            nc.sync.dma_start(xrs[m][:], xr_d[m * 128:(m + 1) * 128, :])
        avTr = avTb[:].rearrange("p (c n) -> p c n", c=8)
        for ib in range(NB):
            osb = p_os.tile([128, D], f32)
            accs = [ps_g.tile([128, 512], f32, name="pg")
                    for k in range(2)]
            for kc in range(4):
                for bank in range(2):
                    nc.tensor.matmul(
                        accs[bank][:],
                        avTr[:, 2 * kc:2 * kc + 2, 128 * ib:128 * (ib + 1)],
                        wo8[2 * bank + kc // 2][:, 2 * (kc % 2):
                                                2 * (kc % 2) + 2, :],
                        start=(kc == 0), stop=(kc == 3), perf_mode=DR)
            for bank in range(2):
                sl = slice(512 * bank, 512 * (bank + 1))
                nc.vector.tensor_scalar_mul(osb[:, sl], accs[bank][:],
                                            1.0 / 1024.0)
                nc.vector.tensor_add(osb[:, sl], osb[:, sl], xrs[ib][:, sl])
            dq().dma_start(out_d[ib * 128:(ib + 1) * 128, :], osb[:])

    nc.compile()
    return nc


def _pos_emb(S_, D_):
    pos_seq = np.arange(S_ - 1, -1, -1.0, dtype=np.float32)
    inv_freq = 1.0 / (10000.0 ** (np.arange(0, D_, 2.0, dtype=np.float32) / D_))
    sinusoid = np.einsum("i,j->ij", pos_seq, inv_freq).astype(np.float32)
    return np.concatenate([np.sin(sinusoid), np.cos(sinusoid)], axis=-1)


def _in_maps(x, Wqkv, Wr, Wo):
    import ml_dtypes
    f8 = ml_dtypes.float8_e4m3fn

    r = _pos_emb(S, D).astype(np.float32) @ np.asarray(Wr, dtype=np.float32)
    r8 = np.ascontiguousarray((32.0 * r.T).astype(f8)).view(np.uint8)
    w8 = np.ascontiguousarray(
        (32.0 * np.asarray(Wqkv, dtype=np.float32)).astype(f8)).view(np.uint8)
    wo = np.ascontiguousarray(
        (32.0 * np.asarray(Wo, dtype=np.float32)).astype(f8)).view(np.uint8)
    i8 = np.zeros((128, 2, 128), dtype=f8)
    for p in range(64):
        for c in range(2):
            i8[p, c, p + 64 * c] = 128.0
            i8[64 + p, c, p + 64 * c] = 128.0
    i8 = np.ascontiguousarray(i8.reshape(128, 256)).view(np.uint8)
    i16 = np.eye(128, dtype=np.float16)

    maps = []
    for b in range(B):
        xb = np.asarray(x[b], dtype=np.float32)
        maps.append({
            "x8": np.ascontiguousarray(xb.T.astype(f8)).view(np.uint8),
            "xr": np.ascontiguousarray(xb.astype(np.float16)),
            "r8": r8, "w8": w8, "wo": wo, "i8": i8, "i16": i16,
        })
    return maps


def kernel(inputs, mask, Wqkv, Wr, Wo):
    from concourse.bass_utils import run_bass_kernel_spmd

    if "nc" not in _CACHED:
        _CACHED["nc"] = _build()
    nc = _CACHED["nc"]
    maps = _in_maps(np.asarray(inputs, dtype=np.float32), Wqkv, Wr, Wo)
    res = run_bass_kernel_spmd(nc, maps, core_ids=list(range(B)))
    out = np.stack([res.results[b]["out"] for b in range(B)], axis=0)
    return out.astype(np.float32)


# revision 38
# speedup vs baseline: 1.0259x; 1.0094x over previous
"""Trainium2 Bass kernel for MultiHeadRelativeSelfAttention (Transformer-XL).

Sharding: data-parallel over batch; 8 cores x 1 batch element, no collectives.

Per-core design (S=1024, H=16, Dh=64; TimelineSim cost model driven):
  * fp8e4(m3) DoubleRow matmuls (0.5 cyc/col) for the q/k/v projections
    (K_eff=256 chunk pairs over host-shipped xT/W in f8), and for the score
    matmuls G=q@r^T and AC=q@k^T (K=64 padded to DR pairs with a zeroed
    SBUF half).  Weights are host-scaled by 32 so f8 stays in normal range;
    scales cancel via a 128x DR identity (BD add), exp scale 0.125/1024 and
    a 1/1024 output rescale.
  * Relative shift: G evicted (x1/128) to f8 and round-tripped through a DRAM
    buffer of row length S+1 (col0=0); reading flat at offset S yields jax's
    _rel_shift exactly.  The BD add into the score PSUM is a DoubleRow
    identity matmul of the f8 bds tile (no extra exp / DVE pass).
  * One exp per score tile ([128,1024] f32 PSUM read across 2 banks) with
    accum_out denominators; reciprocals batched [128,8] per head; probs
    normalized in-place (split GpSimd/DVE).
  * prob^T via DMA XBAR transposes (14ns per 16x128 tile, on the DMA track)
    with a PE-transpose share for balance; PV with v stationary (N=512).
  * Output: (32 avT)(f8) @ (32 Wo)(f8) DoubleRow, x1/1024 rescale + residual
    on DVE; PSUM eviction load is split ACT/DVE; GpSimd handles memsets +
    most prob normalizations.  G runs two heads ahead, bds prefetched one
    head ahead.
"""

import numpy as np
from contextlib import ExitStack

B = 8
D = 1024
H = 16
DH = 64
S = 1024

# routing knobs (tuned against TimelineSim)
G_EVICT_ACT_MOD = (2, 1)     # i%5<2 -> ACT else DVE
NORM_POOL_MOD = (3, 2)       # i%3<2 -> Pool else DVE
TRANSPOSE_PE_MOD = 3         # (2h+half)%MOD==0 -> PE transpose path, else DMA

_CACHED = {}


def _build():
    import concourse.bass as bass
    import concourse.bacc as bacc
    import concourse.tile as tile
    import concourse.mybir as mybir
    from concourse.ap import AP

    f32 = mybir.dt.float32
    f16 = mybir.dt.float16
    f8 = mybir.dt.float8e4
    EXP = mybir.ActivationFunctionType.Exp
    CPY = mybir.ActivationFunctionType.Copy
    DR = mybir.MatmulPerfMode.DoubleRow

    NB = S // 128            # 8 row blocks
    ESC = 0.125 / 1024.0     # exp scale: 1/sqrt(Dh) * 1/(32*32)

    nc = bacc.Bacc("TRN2", target_bir_lowering=False, debug=False)

    x8_d = nc.dram_tensor("x8", [D, S], f8, kind="ExternalInput")      # x^T, f8
    xr_d = nc.dram_tensor("xr", [S, D], f16, kind="ExternalInput")     # residual
    r8_d = nc.dram_tensor("r8", [D, S], f8, kind="ExternalInput")      # (32 pos@Wr)^T
    w8_d = nc.dram_tensor("w8", [D, 3 * D], f8, kind="ExternalInput")  # 32*Wqkv
    wo_d = nc.dram_tensor("wo", [D, D], f8, kind="ExternalInput")      # 32*Wo
    i8_d = nc.dram_tensor("i8", [128, 256], f8, kind="ExternalInput")  # 128*I (DR, both halves)
    i16_d = nc.dram_tensor("i16", [128, 128], f16, kind="ExternalInput")
    out_d = nc.dram_tensor("out", [S, D], f32, kind="ExternalOutput")

    with tile.TileContext(nc) as tc, ExitStack() as es:
        p_x8 = es.enter_context(tc.tile_pool(name="x8", bufs=1))
        p_qk = es.enter_context(tc.tile_pool(name="qk", bufs=1))
        p_r8 = es.enter_context(tc.tile_pool(name="r8", bufs=1))
        p_v = es.enter_context(tc.tile_pool(name="v", bufs=1))
        p_avt = es.enter_context(tc.tile_pool(name="avt", bufs=1))
        p_wst = es.enter_context(tc.tile_pool(name="wst", bufs=2))
        p_i8 = es.enter_context(tc.tile_pool(name="i8", bufs=1))
        p_ga = es.enter_context(tc.tile_pool(name="ga", bufs=3))
        p_bds = es.enter_context(tc.tile_pool(name="bds", bufs=2))  # 2 per name x 2 names
        p_pu = es.enter_context(tc.tile_pool(name="pu", bufs=5))
        p_pt = es.enter_context(tc.tile_pool(name="pt", bufs=3))
        p_dn = es.enter_context(tc.tile_pool(name="dn", bufs=2))
        p_os = es.enter_context(tc.tile_pool(name="os", bufs=2))
        p_y = es.enter_context(tc.tile_pool(name="ydram", bufs=3, space="DRAM"))
        ps_s = es.enter_context(tc.tile_pool(name="pss", bufs=2, space="PSUM"))
        ps_g = es.enter_context(tc.tile_pool(name="psg", bufs=2, space="PSUM"))
        ps_av = es.enter_context(tc.tile_pool(name="psav", bufs=1, space="PSUM"))
        ps_t = es.enter_context(tc.tile_pool(name="pst", bufs=1, space="PSUM"))

        nev = [0]
        ndma = [0]

        def dq():
            ndma[0] += 1
            return nc.sync

        def evict_g(dst, src):
            """G eviction: f32 PSUM -> f8 (x 1/128; 128x identity restores), split ACT/DVE."""
            if nev[0] % G_EVICT_ACT_MOD[0] < G_EVICT_ACT_MOD[1]:
                nc.scalar.activation(dst, src, CPY, scale=1.0 / 128.0)
            else:
                nc.vector.tensor_scalar_mul(dst, src, 1.0 / 128.0)
            nev[0] += 1

        # ---- static loads ----
        x8s = p_x8.tile([128, 8, S], f8)
        nc.sync.dma_start(
            x8s[:], AP(x8_d[:].tensor, 0, [[S, 128], [128 * S, 8], [1, S]]))
        i8s = p_i8.tile([128, 256], f8)
        nc.sync.dma_start(i8s[:], i8_d[:])
        i16s = p_i8.tile([128, 128], f16)
        nc.sync.dma_start(i16s[:], i16_d[:])

        qk = [p_qk.tile([128, 2 * S], f8, name=f"qk{m}") for m in range(16)]
        for m in range(16):
            nc.gpsimd.memset(qk[m][:, S:2 * S], 0.0)
        r8s = [p_r8.tile([128, 2 * S], f8, name=f"r8{m}") for m in range(8)]
        for m in range(8):
            nc.sync.dma_start(r8s[m][:, 0:S], r8_d[m * 128:(m + 1) * 128, :])
            nc.gpsimd.memset(r8s[m][:, S:2 * S], 0.0)
        vsb = [p_v.tile([128, D], f16, name=f"v{m}") for m in range(NB)]
        avTb = p_avt.tile([128, 8 * S], f8)

        def wstream(col0):
            """One-DMA stage of W8 columns [col0, col0+512) as [128, 8, 512]."""
            w = p_wst.tile([128, 8, 512], f8)
            nc.sync.dma_start(
                w[:], AP(w8_d[:].tensor, col0,
                         [[3 * D, 128], [128 * 3 * D, 8], [1, 512]]))
            return w

        # ---- q,k projections (f8-out, DoubleRow, K_eff=256 chunks) ----
        for g in range(4):                       # 4 groups of 512 cols (q,q,k,k)
            w = wstream(512 * g)
            for b in range(4):                   # e-blocks within group
                m = 4 * g + b                    # qk tile index (0-7 q, 8-15 k)
                acc = ps_s.tile([128, S], f32, name="pss")
                for kc in range(4):
                    for bank in range(2):
                        nc.tensor.matmul(
                            acc[:, 512 * bank:512 * (bank + 1)],
                            w[:, 2 * kc:2 * kc + 2, 128 * b:128 * (b + 1)],
                            x8s[:, 2 * kc:2 * kc + 2, 512 * bank:512 * (bank + 1)],
                            start=(kc == 0), stop=(kc == 3), perf_mode=DR)
                nc.vector.tensor_copy(qk[m][:, 0:S], acc[:])

        # ---- v projection (f16-out, [s,e] orientation) ----
        for g in range(2):
            w = wstream(2 * D + 512 * g)
            for sb in range(NB):
                acc = ps_g.tile([128, 512], f32, name="pg")
                for kc in range(4):
                    nc.tensor.matmul(
                        acc[:],
                        x8s[:, 2 * kc:2 * kc + 2, 128 * sb:128 * (sb + 1)],
                        w[:, 2 * kc:2 * kc + 2, :],
                        start=(kc == 0), stop=(kc == 3), perf_mode=DR)
                nc.vector.tensor_copy(vsb[sb][:, 512 * g:512 * (g + 1)], acc[:])

        # ---- attention ----
        def dr2(t, lo, c0, n0, nn):
            """[64, 2, nn] DR operand view of a [128, 2S] tile: c=1 half zero."""
            return t[lo:lo + 64, :].rearrange(
                "p (c n) -> p c n", c=2)[:, :, c0 + n0:c0 + n0 + nn]

        def g_phase(h):
            qt, rt = qk[h // 2], r8s[h // 2]
            lo = 64 * (h % 2)
            y = p_y.tile([S * (S + 1)], f8, name=f"y{h % 3}")
            for half in range(2):
                ga = p_ga.tile([128, 4 * (S + 1)], f8)
                for b4 in range(4):
                    bi = 4 * half + b4
                    base = b4 * (S + 1)
                    nc.gpsimd.memset(ga[:, base:base + 1], 0.0)
                    for bank in range(2):
                        pg = ps_g.tile([128, 512], f32, name="pg")
                        nc.tensor.matmul(
                            pg[:], dr2(qt, lo, 0, 128 * bi, 128),
                            dr2(rt, lo, 0, 512 * bank, 512),
                            start=True, stop=True, perf_mode=DR)
                        evict_g(ga[:, base + 1 + 512 * bank:
                                    base + 1 + 512 * (bank + 1)], pg[:])
                for w2 in range(2):
                    dq().dma_start(
                        AP(y[:].tensor, (half * 512 + 256 * w2) * (S + 1),
                           [[S + 1, 128], [128 * (S + 1), 2], [1, S + 1]]),
                        ga[:, 2 * (S + 1) * w2:2 * (S + 1) * (w2 + 1)]
                        .rearrange("p (c n) -> p c n", c=2))
            return y

        def bds_fetch(h, y):
            lo = 64 * (h % 2)
            tiles = []
            for half in range(2):
                bds = p_bds.tile([128, 8, S], f8, name=f"bds{half}")
                dq().dma_start(
                    bds[lo:lo + 64, :, :],
                    AP(y[:].tensor, S + half * 512 * S,
                       [[S, 64], [64 * S, 8], [1, S]]))
                tiles.append(bds)
            return tiles

        def score_phase(h, bdss):
            qt, kt = qk[h // 2], qk[8 + h // 2]
            lo = 64 * (h % 2)
            den = p_dn.tile([128, 8], f32, name="den")
            rec = p_dn.tile([128, 8], f32, name="rec")
            pus = []
            for half in range(2):
                bds = bdss[half]
                pu = p_pu.tile([128, 4 * S], f16)
                pus.append(pu)
                for b4 in range(4):
                    ib = 4 * half + b4
                    pss = ps_s.tile([128, S], f32, name="pss")
                    for bank in range(2):
                        nc.tensor.matmul(
                            pss[:, 512 * bank:512 * (bank + 1)],
                            i8s[lo:lo + 64, :].rearrange(
                                "p (c n) -> p c n", c=2),
                            bds[lo:lo + 64, 2 * b4:2 * b4 + 2,
                                512 * bank:512 * (bank + 1)],
                            start=True, stop=False, perf_mode=DR)
                    for bank in range(2):
                        nc.tensor.matmul(
                            pss[:, 512 * bank:512 * (bank + 1)],
                            dr2(qt, lo, 0, 128 * ib, 128),
                            dr2(kt, lo, 0, 512 * bank, 512),
                            start=False, stop=True, perf_mode=DR)
                    nc.scalar.activation(
                        pu[:, S * b4:S * (b4 + 1)], pss[:], EXP, scale=ESC,
                        accum_out=den[:, ib:ib + 1])
            nc.vector.reciprocal(rec[:], den[:])

            for half in range(2):
                pu = pus[half]
                pt = p_pt.tile([128, 32, 128], f16)
                for b4 in range(4):
                    ib = 4 * half + b4
                    if (2 * ib + h) % NORM_POOL_MOD[0] < NORM_POOL_MOD[1]:
                        eng = nc.gpsimd
                    else:
                        eng = nc.vector
                    eng.tensor_scalar_mul(
                        pu[:, S * b4:S * (b4 + 1)], pu[:, S * b4:S * (b4 + 1)],
                        rec[:, ib:ib + 1])
                if TRANSPOSE_PE_MOD and (2 * h + half) % TRANSPOSE_PE_MOD == 0:
                    # PE transpose path: per ib, 8 [128,128] transposes -> psum
                    for b4 in range(4):
                        ptp = ps_t.tile([128, S], f16, name="ptp")
                        for jc in range(8):
                            nc.tensor.transpose(
                                ptp[:, 128 * jc:128 * (jc + 1)],
                                pu[:, S * b4 + 128 * jc:S * b4 + 128 * (jc + 1)],
                                i16s[:])
                        nc.vector.tensor_copy(
                            pt[:, 8 * b4:8 * (b4 + 1), :].rearrange(
                                "p a b -> p (a b)"),
                            ptp[:])
                else:
                    dq().dma_start_transpose(pt[:], pu[:])
                # PV: av[dh, (ib4, i)] accumulate over j chunks
                pav = ps_av.tile([64, 512], f32, name="pav")
                pt_r = pt[:].rearrange("p (b j) i -> p j b i", j=8)
                for jc in range(8):
                    nc.tensor.matmul(
                        pav[:], vsb[jc][:, DH * h:DH * (h + 1)],
                        pt_r[:, jc:jc + 1, :, :],
                        start=(jc == 0), stop=(jc == 7))
                nc.vector.tensor_copy(
                    avTb[lo:lo + 64, (h // 2) * S + 512 * half:
                         (h // 2) * S + 512 * (half + 1)], pav[:])

        ys = {0: g_phase(0), 1: g_phase(1)}
        bq = {0: bds_fetch(0, ys.pop(0))}
        for h in range(H):
            if h + 2 < H:
                ys[h + 2] = g_phase(h + 2)
            if h + 1 < H:
                bq[h + 1] = bds_fetch(h + 1, ys.pop(h + 1))
            score_phase(h, bq.pop(h))

        # ---- out = (32 avT)^T @ (32 Wo) / 1024 + x  (DoubleRow f8) ----
        wo8 = []
        for t in range(4):                      # (bank, kc-half) quarter tiles
            bank, kh = t // 2, t % 2
            wt = p_r8.tile([128, 4, 512], f8, name=f"r8{t}")
            nc.sync.dma_start(
                wt[:], AP(wo_d[:].tensor, 512 * kh * D + 512 * bank,
                          [[D, 128], [128 * D, 4], [1, 512]]))
            wo8.append(wt)
        xrs = [p_qk.tile([128, D], f16, name=f"qk{m}") for m in range(8)]
        for m in range(8):
            nc.sync.dma_start(xrs[m][:], xr_d[m * 128:(m + 1) * 128, :])
        avTr = avTb[:].rearrange("p (c n) -> p c n", c=8)
        for ib in range(NB):
            osb = p_os.tile([128, D], f32)
            accs = [ps_g.tile([128, 512], f32, name="pg")
                    for k in range(2)]
            for kc in range(4):
                for bank in range(2):
                    nc.tensor.matmul(
                        accs[bank][:],
                        avTr[:, 2 * kc:2 * kc + 2, 128 * ib:128 * (ib + 1)],
                        wo8[2 * bank + kc // 2][:, 2 * (kc % 2):
                                                2 * (kc % 2) + 2, :],
                        start=(kc == 0), stop=(kc == 3), perf_mode=DR)
            for bank in range(2):
                sl = slice(512 * bank, 512 * (bank + 1))
                nc.vector.tensor_scalar_mul(osb[:, sl], accs[bank][:],
                                            1.0 / 1024.0)
                nc.vector.tensor_add(osb[:, sl], osb[:, sl], xrs[ib][:, sl])
            dq().dma_start(out_d[ib * 128:(ib + 1) * 128, :], osb[:])

    nc.compile()
    return nc


def _pos_emb(S_, D_):
    pos_seq = np.arange(S_ - 1, -1, -1.0, dtype=np.float32)
    inv_freq = 1.0 / (10000.0 ** (np.arange(0, D_, 2.0, dtype=np.float32) / D_))
    sinusoid = np.einsum("i,j->ij", pos_seq, inv_freq).astype(np.float32)
    return np.concatenate([np.sin(sinusoid), np.cos(sinusoid)], axis=-1)


def _in_maps(x, Wqkv, Wr, Wo):
    import ml_dtypes
    f8 = ml_dtypes.float8_e4m3fn

    r = _pos_emb(S, D).astype(np.float32) @ np.asarray(Wr, dtype=np.float32)
    r8 = np.ascontiguousarray((32.0 * r.T).astype(f8)).view(np.uint8)
    w8 = np.ascontiguousarray(
        (32.0 * np.asarray(Wqkv, dtype=np.float32)).astype(f8)).view(np.uint8)
    wo = np.ascontiguousarray(
        (32.0 * np.asarray(Wo, dtype=np.float32)).astype(f8)).view(np.uint8)
    i8 = np.zeros((128, 2, 128), dtype=f8)
    for p in range(64):
        for c in range(2):
            i8[p, c, p + 64 * c] = 128.0
            i8[64 + p, c, p + 64 * c] = 128.0
    i8 = np.ascontiguousarray(i8.reshape(128, 256)).view(np.uint8)
    i16 = np.eye(128, dtype=np.float16)

    maps = []
    for b in range(B):
        xb = np.asarray(x[b], dtype=np.float32)
        maps.append({
            "x8": np.ascontiguousarray(xb.T.astype(f8)).view(np.uint8),
            "xr": np.ascontiguousarray(xb.astype(np.float16)),
            "r8": r8, "w8": w8, "wo": wo, "i8": i8, "i16": i16,
        })
    return maps


def kernel(inputs, mask, Wqkv, Wr, Wo):
    from concourse.bass_utils import run_bass_kernel_spmd

    if "nc" not in _CACHED:
        _CACHED["nc"] = _build()
    nc = _CACHED["nc"]
    maps = _in_maps(np.asarray(inputs, dtype=np.float32), Wqkv, Wr, Wo)
    res = run_bass_kernel_spmd(nc, maps, core_ids=list(range(B)))
    out = np.stack([res.results[b]["out"] for b in range(B)], axis=0)
    return out.astype(np.float32)
